# revision 55
# baseline (speedup 1.0000x reference)
# ARFSA attention kernel for 8 TRN2 NeuronCores (Bass/Tile), v6.
#
# Reference computation (per batch b, channel c):
#   q = Wq x + bq ; k = Wk x + bk ; v = Wv x + bv          (1x1 convs)
#   att = softmax_flat( q @ (k + P)^T )                    (P = pos_code)
#   out = att * v
#
# v6 design (data-parallel over batch, 4 per core):
#   * ONE fused projection matmul per x-chunk: N=192 (q|k+P|v) with a
#     single LDWEIGHTS -- the LDW+MM pair cadence (~107ns, LDW-paced)
#     was the v5 PE bottleneck; fusing V into the QK stream removes 128
#     pairs/batch and raises PE-array duty (HAM warm chance).
#   * Everything downstream is h-major: qkv_sb [w, h, 192c] (contiguous
#     evictions), E [w, h, c] via att-psum [128, 128h, 4c] (strided
#     column MM writes, measured mild), V slice of qkv_sb.
#   * softmax denominators via a pairwise-halving TT tree over the h
#     axis (bf16, 2x_1P) instead of 1x tensor_reduce.
#   * sinv applied in pass2 stage2 through an outer-dim broadcast view
#     (innermost c stride-1 keeps the DVE 2x_1P mode).
#   * PSUM: proj [128, 4, 256] (2 banks, chunks at 1KB offsets) x2,
#     att [128, 128, 4] (1 bank, strided cols span <=2KB) x2.
#
# Layouts (per core):
#   xa   DRAM in  [4, 66, 16384] fp16   rows 0..63 = x, row 64 = 1.0 (bias),
#                                       row 65 = P.flatten() (K-only via waug)
#   waug DRAM in  [66, 192] fp16        cols 0:64 Wq^T | 64:128 Wk^T | 128:192 Wv^T
#   out  DRAM out [4, 128(w), 128(h), 64(c)] bf16  (host -> [b,c,h,w])

import sys

if "/opt/trn_rl_repo" not in sys.path:
    sys.path.insert(0, "/opt/trn_rl_repo")

import numpy as np
from contextlib import ExitStack

import concourse.bass as bass
import concourse.tile as tile
from concourse import bacc, mybir
from concourse.bass_utils import run_bass_kernel_spmd

N_CORES = 8
B_LOC = 4            # 32 batches / 8 cores
C = 64               # out channels
F = 128              # feature map size
S = F * F            # 16384 positions
QUART = S // 4       # x loaded in four quarters
XBUFS = 5            # physical buffers in the x quarter pool

FP16 = mybir.dt.float16
BF16 = mybir.dt.bfloat16
F32 = mybir.dt.float32

_BUILT = {}

# Engine split knobs.
# 32 projection evictions per batch: which go to ScalarE (rest VectorE).
# First 8 groups all on Act: gives the PE eviction runway at iteration
# start while the DVE finishes the previous batch's pass2 chain.
EVICT_ON_ACT = set(range(8)) | set(range(8, 32, 2))          # 20 of 32
# 8 pass2 h-groups: which stage-1 calls run on GpSimd (rest VectorE).
PASS2A_ON_GPS = {3, 7}


def _build_bass():
    nc = bacc.Bacc("TRN2", target_bir_lowering=False, debug=False)

    xa = nc.declare_dram_parameter("xa", [B_LOC, 66, S], FP16, isOutput=False)
    waug = nc.declare_dram_parameter("waug", [66, 192], FP16, isOutput=False)
    out = nc.declare_dram_parameter("out", [B_LOC, F, F, C], BF16, isOutput=True)

    with ExitStack() as ctx:
        tc = ctx.enter_context(tile.TileContext(nc))

        const = ctx.enter_context(tc.tile_pool(name="const", bufs=1))
        xpool = ctx.enter_context(tc.tile_pool(name="xpool", bufs=XBUFS))
        qkvpool = ctx.enter_context(tc.tile_pool(name="qkvpool", bufs=2))
        epool = ctx.enter_context(tc.tile_pool(name="epool", bufs=2))
        treepool = ctx.enter_context(tc.tile_pool(name="treepool", bufs=1))
        rpool = ctx.enter_context(tc.tile_pool(name="rpool", bufs=2))
        tpool = ctx.enter_context(tc.tile_pool(name="tpool", bufs=2))
        opool = ctx.enter_context(tc.tile_pool(name="opool", bufs=3))
        ps = ctx.enter_context(tc.tile_pool(name="ps", bufs=3, space="PSUM"))
        psa = ctx.enter_context(tc.tile_pool(name="psa", bufs=2, space="PSUM"))

        waug_sb = const.tile([66, 192], FP16, tag="waug")
        nc.sync.dma_start(out=waug_sb[:], in_=waug[:, :])
        ones_sb = const.tile([128, 128], BF16, tag="ones")
        nc.gpsimd.memset(ones_sb[:], 1.0)

        st = {}   # per-batch pipeline state
        xcount = [0]

        def emit_xload(b, q):
            # x quarters padded to K=128: rows 66:128 are zeroed once per
            # physical buffer (round-robin tag rotation); the padded waug
            # rows are zero so the extra rows contribute nothing, the
            # memset just guarantees finite values.  Full 128-row
            # stationaries keep the PE array fully active (HAM warm-up).
            x_t = xpool.tile([66, QUART], FP16, tag="xt", name=f"xt_{b}_{q}")
            for xc in range(2):
                nc.sync.dma_start(
                    out=x_t[:, xc * 2048:(xc + 1) * 2048],
                    in_=xa[b, :, q * QUART + xc * 2048:
                           q * QUART + (xc + 1) * 2048])
            st.setdefault(b, {})[f"x{q}"] = x_t

        def emit_proj_group(b, g):
            # 4 chunks, one fused N=192 matmul each (chunks at 1KB PSUM
            # offsets so each output stays inside one bank).
            s = st[b]
            if g == 0:
                s["qkv"] = qkvpool.tile([128, F, 192], FP16, tag="qkv",
                                        name=f"qkv_{b}")  # [w, h, q|kp|v]
            x_t = s[f"x{g // 8}"]
            goff = (g % 8) * 4
            pp = ps.tile([128, 4, 256], F32, tag="ps", name=f"pp_{b}_{g}")
            for jj in range(4):
                xs = x_t[:, (goff + jj) * F:(goff + jj + 1) * F]
                nc.tensor.matmul(pp[:, jj, 0:192], lhsT=xs,
                                 rhs=waug_sb[:, 0:192], start=True, stop=True)
            # batch 0 has no trailing pass2 chain on the DVE -- plain
            # alternation paces evictions fastest there.
            on_act = (g % 2 == 0) if b == 0 else (g in EVICT_ON_ACT)
            eng = nc.scalar.copy if on_act else nc.vector.tensor_copy
            eng(s["qkv"][:, g * 4:(g + 1) * 4, :], pp[:, :, 0:192])

        def emit_att_group(b, cg):
            # 4 channels: att matmuls with strided column writes into
            # [128, 128h, 4c]; exp evicts h-major (8B-run strided dst).
            s = st[b]
            if cg == 0:
                s["e"] = epool.tile([128, F, C], BF16, tag="e", name=f"e_{b}")
            c0 = cg * 4
            at = psa.tile([128, F, 4], F32, tag="psa", name=f"at_{b}_{cg}")
            for cc in range(4):
                c = c0 + cc
                nc.tensor.matmul(
                    at[:, :, cc],
                    lhsT=s["qkv"][:, :, 64 + c],   # (K+P)^T tile [w, v]
                    rhs=s["qkv"][:, :, c],         # Q^T tile [w, h]
                    start=True, stop=True,
                )
            nc.scalar.activation(
                s["e"][:, :, c0:c0 + 4], at[:, :, :],
                mybir.ActivationFunctionType.Exp,
            )

        def emit_tree_quarter(b, q):
            # r[w, c] = sum_h E[w, h, c] via pairwise halving (2x_1P
            # TTs), for the 16-channel quarter q (emitted as soon as
            # those channels' att groups are done -> only the last
            # quarter's latency is exposed at the iteration boundary).
            s = st[b]
            if q == 0:
                s["tree"] = treepool.tile([128, F, C], BF16, tag="tree",
                                          name=f"tr_{b}")
            t = s["tree"]
            e = s["e"]
            cs = slice(q * 16, (q + 1) * 16)
            with nc.allow_low_precision("bf16 softmax denominators"):
                nc.vector.tensor_add(t[:, 0:64, cs], e[:, 0:64, cs],
                                     e[:, 64:128, cs])
                o = 0
                w = 32
                while w >= 1:
                    nc.vector.tensor_add(t[:, o + 2 * w:o + 3 * w, cs],
                                         t[:, o:o + w, cs],
                                         t[:, o + w:o + 2 * w, cs])
                    o += 2 * w
                    w //= 2
                # final per-(w,c) sums land in row 126

        def emit_sinv(b):
            s = st[b]
            t = s["tree"]
            spt = psa.tile([128, F, 4], F32, tag="psa", name=f"sp_{b}")
            sp = spt[:, 0:64, 0]
            nc.tensor.matmul(sp, lhsT=ones_sb[:], rhs=t[:, 126, :],
                             start=True, stop=True)
            sinv = rpool.tile([128, C], F32, tag="sinv", name=f"sinv_{b}")
            nc.vector.reciprocal(sinv[:, :], sp)
            sb16 = rpool.tile([128, C], BF16, tag="sb16", name=f"sb16_{b}")
            nc.vector.tensor_copy(sb16[:, :], sinv[:, :])
            s["sb16"] = sb16

        def emit_sinv_quarter(b, q):
            # Per-channel softmax independence: quarter q's denominators
            # finish as soon as its tree does (used for the last batch to
            # overlap the softmax/pass2 chain with its own att phase).
            s = st[b]
            t = s["tree"]
            if q == 0:
                s["sinv"] = rpool.tile([128, C], F32, tag="sinv",
                                       name=f"sinv_{b}")
                s["sb16"] = rpool.tile([128, C], BF16, tag="sb16",
                                       name=f"sb16_{b}")
            # scratch from the proj psum pool, NOT psa: an psa-pool tile
            # here makes later att groups wait on the sinv chain (the
            # pool rotation serializes allocation on the consumer).
            spt = ps.tile([128, 4, 256], F32, tag="ps", name=f"sp_{b}_{q}")
            sp = spt[:, 0, 0:16]
            nc.tensor.matmul(sp, lhsT=ones_sb[:],
                             rhs=t[:, 126, q * 16:(q + 1) * 16],
                             start=True, stop=True)
            nc.vector.reciprocal(s["sinv"][:, q * 16:(q + 1) * 16], sp)
            nc.vector.tensor_copy(s["sb16"][:, q * 16:(q + 1) * 16],
                                  s["sinv"][:, q * 16:(q + 1) * 16])

        def emit_pass2q(b, q):
            # pass2 for channel quarter q over all h; the DMA dst is a
            # strided slice of the [b, w, h, c] output (32B runs) -- only
            # used in the att-only last iteration where DMA has slack.
            s = st[b]
            cs = slice(q * 16, (q + 1) * 16)
            tt = tpool.tile([128, F, 16], BF16, tag="ttq",
                            name=f"ttq_{b}_{q}", bufs=1)
            ot = opool.tile([128, F, 16], BF16, tag="otq",
                            name=f"otq_{b}_{q}", bufs=1)
            nc.vector.tensor_mul(tt[:, :, :], s["e"][:, :, cs],
                                 s["qkv"][:, :, 128 + q * 16:128 + (q + 1) * 16])
            sv = s["sb16"][:, cs].unsqueeze(1).broadcast_to([128, F, 16])
            nc.vector.tensor_mul(ot[:, :, :], tt[:, :, :], sv)
            nc.sync.dma_start(out=out[b, :, :, cs], in_=ot[:])

        def emit_pass2(b, hg):
            # 16 h-rows: t = E*V (2x_1P), ot = t * sinv_bcast (2x_1P).
            s = st[b]
            h0 = hg * 16
            tt = tpool.tile([128, 16, C], BF16, tag="tt", name=f"tt_{b}_{hg}")
            ot = opool.tile([128, 16, C], BF16, tag="ot", name=f"ot_{b}_{hg}")
            e_sl = s["e"][:, h0:h0 + 16, :]
            v_sl = s["qkv"][:, h0:h0 + 16, 128:192]
            # last batch: GpSimd's slow TT latency would sit exposed in
            # the kernel tail -- keep it all on the DVE there.
            use_gps = hg in PASS2A_ON_GPS and b != B_LOC - 1
            eng1 = nc.gpsimd if use_gps else nc.vector
            eng1.tensor_mul(tt[:, :, :], e_sl, v_sl)
            sv = s["sb16"][:, :].unsqueeze(1).broadcast_to([128, 16, C])
            nc.vector.tensor_mul(ot[:, :, :], tt[:, :, :], sv)
            nc.sync.dma_start(out=out[b, :, h0:h0 + 16, :], in_=ot[:])

        # ---- software pipeline ----
        # iteration i: proj(i) with att(i-1) interleaved, then sinv /
        # pass2 of (i-1) trailing on the DVE.
        for q in range(4):
            emit_xload(0, q)
        for i in range(B_LOC + 1):
            p = i if i < B_LOC else None            # projection batch
            c = i - 1 if i >= 1 else None           # att + pass2 batch

            last = c == B_LOC - 1
            for g in range(32):
                if p is not None:
                    emit_proj_group(p, g)
                    if g % 8 == 4 and p + 1 < B_LOC:
                        emit_xload(p + 1, g // 8)
                if c is not None and g % 2 == 1:
                    emit_att_group(c, g // 2)
                    if g % 8 == 7:
                        emit_tree_quarter(c, g // 8)
                # last batch: per-quarter softmax chain overlaps its own
                # att phase (no evictions to displace in this iteration).
                if last and g in (13, 21, 29):
                    emit_sinv_quarter(c, {13: 0, 21: 1, 29: 2}[g])
                if last and g in (15, 23, 31):
                    emit_pass2q(c, {15: 0, 23: 1, 31: 2}[g])
            if c is not None and not last:
                emit_sinv(c)
                for hg in range(8):
                    emit_pass2(c, hg)
            if c is not None and last:
                emit_sinv_quarter(c, 3)
                emit_pass2q(c, 3)

    nc.compile()
    return nc


def _get_built():
    if "nc" not in _BUILT:
        _BUILT["nc"] = _build_bass()
    return _BUILT["nc"]


def _prep_inputs(x, wq, bq, wk, bk, wv, bv, pos_code):
    x = np.asarray(x, np.float32)
    pos = np.asarray(pos_code, np.float32)[0]          # identical across channels
    waug = np.zeros([66, 192], np.float32)
    waug[0:64, 0:64] = np.asarray(wq, np.float32).T
    waug[0:64, 64:128] = np.asarray(wk, np.float32).T
    waug[0:64, 128:192] = np.asarray(wv, np.float32).T
    waug[64, 0:64] = np.asarray(bq, np.float32)
    waug[64, 64:128] = np.asarray(bk, np.float32)
    waug[64, 128:192] = np.asarray(bv, np.float32)
    waug[65, 64:128] = 1.0                             # P-row hits K channels only
    waug16 = waug.astype(np.float16)

    pflat16 = pos.reshape(-1).astype(np.float16)
    xf = x.reshape(x.shape[0], x.shape[1], S)
    in_maps = []
    for core in range(N_CORES):
        xs = xf[core * B_LOC:(core + 1) * B_LOC]
        xa = np.empty([B_LOC, 66, S], np.float16)
        xa[:, 0:64] = xs.astype(np.float16)
        xa[:, 64] = np.float16(1.0)
        xa[:, 65] = pflat16[None, :]
        in_maps.append({"xa": xa, "waug": waug16})
    return in_maps


LAST_RESULTS = None


def kernel(x, wq, bq, wk, bk, wv, bv, pos_code, _trace=False):
    global LAST_RESULTS
    in_maps = _prep_inputs(x, wq, bq, wk, bk, wv, bv, pos_code)
    nc = _get_built()
    res = run_bass_kernel_spmd(nc, in_maps, core_ids=list(range(N_CORES)),
                               trace=_trace)
    LAST_RESULTS = res
    outs = []
    for core in range(N_CORES):
        o = np.asarray(res.results[core]["out"])       # [4, w, h, c] bf16
        outs.append(np.transpose(o.astype(np.float32), (0, 3, 2, 1)))
    return np.concatenate(outs, axis=0)


# revision 56
# speedup vs baseline: 1.3237x; 1.3237x over previous
# ARFSA attention kernel for 8 TRN2 NeuronCores (Bass/Tile), v6.
#
# Reference computation (per batch b, channel c):
#   q = Wq x + bq ; k = Wk x + bk ; v = Wv x + bv          (1x1 convs)
#   att = softmax_flat( q @ (k + P)^T )                    (P = pos_code)
#   out = att * v
#
# v6 design (data-parallel over batch, 4 per core):
#   * ONE fused projection matmul per x-chunk: N=192 (q|k+P|v) with a
#     single LDWEIGHTS -- the LDW+MM pair cadence (~107ns, LDW-paced)
#     was the v5 PE bottleneck; fusing V into the QK stream removes 128
#     pairs/batch and raises PE-array duty (HAM warm chance).
#   * Everything downstream is h-major: qkv_sb [w, h, 192c] (contiguous
#     evictions), E [w, h, c] via att-psum [128, 128h, 4c] (strided
#     column MM writes, measured mild), V slice of qkv_sb.
#   * softmax denominators via a pairwise-halving TT tree over the h
#     axis (bf16, 2x_1P) instead of 1x tensor_reduce.
#   * sinv applied in pass2 stage2 through an outer-dim broadcast view
#     (innermost c stride-1 keeps the DVE 2x_1P mode).
#   * PSUM: proj [128, 4, 256] (2 banks, chunks at 1KB offsets) x2,
#     att [128, 128, 4] (1 bank, strided cols span <=2KB) x2.
#
# Layouts (per core):
#   xa   DRAM in  [4, 66, 16384] fp16   rows 0..63 = x, row 64 = 1.0 (bias),
#                                       row 65 = P.flatten() (K-only via waug)
#   waug DRAM in  [66, 192] fp16        cols 0:64 Wq^T | 64:128 Wk^T | 128:192 Wv^T
#   out  DRAM out [4, 128(w), 128(h), 64(c)] bf16  (host -> [b,c,h,w])

import sys

if "/opt/trn_rl_repo" not in sys.path:
    sys.path.insert(0, "/opt/trn_rl_repo")

import numpy as np
from contextlib import ExitStack

import concourse.bass as bass
import concourse.tile as tile
from concourse import bacc, mybir
from concourse.bass_utils import run_bass_kernel_spmd

N_CORES = 8
B_LOC = 4            # 32 batches / 8 cores
C = 64               # out channels
F = 128              # feature map size
S = F * F            # 16384 positions
QUART = S // 4       # x loaded in four quarters
XBUFS = 5            # physical buffers in the x quarter pool

FP16 = mybir.dt.float16
BF16 = mybir.dt.bfloat16
F32 = mybir.dt.float32

_BUILT = {}

# Engine split knobs.
# 32 projection evictions per batch: which go to ScalarE (rest VectorE).
# First 8 groups all on Act: gives the PE eviction runway at iteration
# start while the DVE finishes the previous batch's pass2 chain.
EVICT_ON_ACT = set(range(8)) | set(range(8, 32, 2))          # 20 of 32
# 8 pass2 h-groups: which stage-1 calls run on GpSimd (rest VectorE).
PASS2A_ON_GPS = {3, 7}


def _build_bass():
    nc = bacc.Bacc("TRN2", target_bir_lowering=False, debug=False)

    xa = nc.declare_dram_parameter("xa", [B_LOC, 66, S], FP16, isOutput=False)
    waug = nc.declare_dram_parameter("waug", [66, 192], FP16, isOutput=False)
    out = nc.declare_dram_parameter("out", [B_LOC, F, F, C], BF16, isOutput=True)

    with ExitStack() as ctx:
        tc = ctx.enter_context(tile.TileContext(nc))

        const = ctx.enter_context(tc.tile_pool(name="const", bufs=1))
        xpool = ctx.enter_context(tc.tile_pool(name="xpool", bufs=XBUFS))
        qkvpool = ctx.enter_context(tc.tile_pool(name="qkvpool", bufs=2))
        epool = ctx.enter_context(tc.tile_pool(name="epool", bufs=2))
        treepool = ctx.enter_context(tc.tile_pool(name="treepool", bufs=1))
        rpool = ctx.enter_context(tc.tile_pool(name="rpool", bufs=2))
        tpool = ctx.enter_context(tc.tile_pool(name="tpool", bufs=2))
        opool = ctx.enter_context(tc.tile_pool(name="opool", bufs=3))
        ps = ctx.enter_context(tc.tile_pool(name="ps", bufs=3, space="PSUM"))
        psa = ctx.enter_context(tc.tile_pool(name="psa", bufs=2, space="PSUM"))

        waug_sb = const.tile([66, 192], FP16, tag="waug")
        nc.sync.dma_start(out=waug_sb[:], in_=waug[:, :])
        ones_sb = const.tile([128, 128], BF16, tag="ones")
        nc.gpsimd.memset(ones_sb[:], 1.0)

        st = {}   # per-batch pipeline state
        xcount = [0]

        def emit_xload(b, q):
            # x quarters padded to K=128: rows 66:128 are zeroed once per
            # physical buffer (round-robin tag rotation); the padded waug
            # rows are zero so the extra rows contribute nothing, the
            # memset just guarantees finite values.  Full 128-row
            # stationaries keep the PE array fully active (HAM warm-up).
            x_t = xpool.tile([66, QUART], FP16, tag="xt", name=f"xt_{b}_{q}")
            for xc in range(2):
                nc.sync.dma_start(
                    out=x_t[:, xc * 2048:(xc + 1) * 2048],
                    in_=xa[b, :, q * QUART + xc * 2048:
                           q * QUART + (xc + 1) * 2048])
            st.setdefault(b, {})[f"x{q}"] = x_t

        def emit_proj_group(b, g):
            # 4 chunks, one fused N=192 matmul each (chunks at 1KB PSUM
            # offsets so each output stays inside one bank).
            s = st[b]
            if g == 0:
                s["qkv"] = qkvpool.tile([128, F, 192], FP16, tag="qkv",
                                        name=f"qkv_{b}")  # [w, h, q|kp|v]
            x_t = s[f"x{g // 8}"]
            goff = (g % 8) * 4
            pp = ps.tile([128, 4, 256], F32, tag="ps", name=f"pp_{b}_{g}")
            for jj in range(4):
                xs = x_t[:, (goff + jj) * F:(goff + jj + 1) * F]
                nc.tensor.matmul(pp[:, jj, 0:192], lhsT=xs,
                                 rhs=waug_sb[:, 0:192], start=True, stop=True)
            # batch 0 has no trailing pass2 chain on the DVE -- plain
            # alternation paces evictions fastest there.
            on_act = (g % 2 == 0) if b == 0 else (g in EVICT_ON_ACT)
            eng = nc.scalar.copy if on_act else nc.vector.tensor_copy
            eng(s["qkv"][:, g * 4:(g + 1) * 4, :], pp[:, :, 0:192])

        def emit_att_group(b, cg):
            # 4 channels: att matmuls with strided column writes into
            # [128, 128h, 4c]; exp evicts h-major (8B-run strided dst).
            s = st[b]
            if cg == 0:
                s["e"] = epool.tile([128, F, C], BF16, tag="e", name=f"e_{b}")
            c0 = cg * 4
            at = psa.tile([128, F, 4], F32, tag="psa", name=f"at_{b}_{cg}")
            for cc in range(4):
                c = c0 + cc
                nc.tensor.matmul(
                    at[:, :, cc],
                    lhsT=s["qkv"][:, :, 64 + c],   # (K+P)^T tile [w, v]
                    rhs=s["qkv"][:, :, c],         # Q^T tile [w, h]
                    start=True, stop=True,
                )
            nc.scalar.activation(
                s["e"][:, :, c0:c0 + 4], at[:, :, :],
                mybir.ActivationFunctionType.Exp,
            )

        def emit_tree_quarter(b, q):
            # r[w, c] = sum_h E[w, h, c] via pairwise halving (2x_1P
            # TTs), for the 16-channel quarter q (emitted as soon as
            # those channels' att groups are done -> only the last
            # quarter's latency is exposed at the iteration boundary).
            s = st[b]
            if q == 0:
                s["tree"] = treepool.tile([128, F, C], BF16, tag="tree",
                                          name=f"tr_{b}")
            t = s["tree"]
            e = s["e"]
            cs = slice(q * 16, (q + 1) * 16)
            with nc.allow_low_precision("bf16 softmax denominators"):
                nc.vector.tensor_add(t[:, 0:64, cs], e[:, 0:64, cs],
                                     e[:, 64:128, cs])
                o = 0
                w = 32
                while w >= 1:
                    nc.vector.tensor_add(t[:, o + 2 * w:o + 3 * w, cs],
                                         t[:, o:o + w, cs],
                                         t[:, o + w:o + 2 * w, cs])
                    o += 2 * w
                    w //= 2
                # final per-(w,c) sums land in row 126

        def emit_sinv(b):
            s = st[b]
            t = s["tree"]
            spt = psa.tile([128, F, 4], F32, tag="psa", name=f"sp_{b}")
            sp = spt[:, 0:64, 0]
            nc.tensor.matmul(sp, lhsT=ones_sb[:], rhs=t[:, 126, :],
                             start=True, stop=True)
            sinv = rpool.tile([128, C], F32, tag="sinv", name=f"sinv_{b}")
            nc.vector.reciprocal(sinv[:, :], sp)
            sb16 = rpool.tile([128, C], BF16, tag="sb16", name=f"sb16_{b}")
            nc.vector.tensor_copy(sb16[:, :], sinv[:, :])
            s["sb16"] = sb16

        def emit_pass2(b, hg):
            # 16 h-rows: t = E*V (2x_1P), ot = t * sinv_bcast (2x_1P).
            s = st[b]
            h0 = hg * 16
            tt = tpool.tile([128, 16, C], BF16, tag="tt", name=f"tt_{b}_{hg}")
            ot = opool.tile([128, 16, C], BF16, tag="ot", name=f"ot_{b}_{hg}")
            e_sl = s["e"][:, h0:h0 + 16, :]
            v_sl = s["qkv"][:, h0:h0 + 16, 128:192]
            # last batch: GpSimd's slow TT latency would sit exposed in
            # the kernel tail -- keep it all on the DVE there.
            use_gps = hg in PASS2A_ON_GPS and b != B_LOC - 1
            eng1 = nc.gpsimd if use_gps else nc.vector
            eng1.tensor_mul(tt[:, :, :], e_sl, v_sl)
            sv = s["sb16"][:, :].unsqueeze(1).broadcast_to([128, 16, C])
            nc.vector.tensor_mul(ot[:, :, :], tt[:, :, :], sv)
            nc.sync.dma_start(out=out[b, :, h0:h0 + 16, :], in_=ot[:])

        # ---- software pipeline ----
        # iteration i: proj(i) with att(i-1) interleaved, then sinv /
        # pass2 of (i-1) trailing on the DVE.
        for q in range(4):
            emit_xload(0, q)
        for i in range(B_LOC + 1):
            p = i if i < B_LOC else None            # projection batch
            c = i - 1 if i >= 1 else None           # att + pass2 batch

            for g in range(32):
                if p is not None:
                    emit_proj_group(p, g)
                    if g % 8 == 4 and p + 1 < B_LOC:
                        emit_xload(p + 1, g // 8)
                if c is not None and g % 2 == 1:
                    emit_att_group(c, g // 2)
                    if g % 8 == 7:
                        emit_tree_quarter(c, g // 8)
            if c is not None:
                emit_sinv(c)
                for hg in range(8):
                    emit_pass2(c, hg)

    nc.compile()
    return nc


def _get_built():
    if "nc" not in _BUILT:
        _BUILT["nc"] = _build_bass()
    return _BUILT["nc"]


def _prep_inputs(x, wq, bq, wk, bk, wv, bv, pos_code):
    x = np.asarray(x, np.float32)
    pos = np.asarray(pos_code, np.float32)[0]          # identical across channels
    waug = np.zeros([66, 192], np.float32)
    waug[0:64, 0:64] = np.asarray(wq, np.float32).T
    waug[0:64, 64:128] = np.asarray(wk, np.float32).T
    waug[0:64, 128:192] = np.asarray(wv, np.float32).T
    waug[64, 0:64] = np.asarray(bq, np.float32)
    waug[64, 64:128] = np.asarray(bk, np.float32)
    waug[64, 128:192] = np.asarray(bv, np.float32)
    waug[65, 64:128] = 1.0                             # P-row hits K channels only
    waug16 = waug.astype(np.float16)

    pflat16 = pos.reshape(-1).astype(np.float16)
    xf = x.reshape(x.shape[0], x.shape[1], S)
    in_maps = []
    for core in range(N_CORES):
        xs = xf[core * B_LOC:(core + 1) * B_LOC]
        xa = np.empty([B_LOC, 66, S], np.float16)
        xa[:, 0:64] = xs.astype(np.float16)
        xa[:, 64] = np.float16(1.0)
        xa[:, 65] = pflat16[None, :]
        in_maps.append({"xa": xa, "waug": waug16})
    return in_maps


LAST_RESULTS = None


def kernel(x, wq, bq, wk, bk, wv, bv, pos_code, _trace=False):
    global LAST_RESULTS
    in_maps = _prep_inputs(x, wq, bq, wk, bk, wv, bv, pos_code)
    nc = _get_built()
    res = run_bass_kernel_spmd(nc, in_maps, core_ids=list(range(N_CORES)),
                               trace=_trace)
    LAST_RESULTS = res
    outs = []
    for core in range(N_CORES):
        o = np.asarray(res.results[core]["out"])       # [4, w, h, c] bf16
        outs.append(np.transpose(o.astype(np.float32), (0, 3, 2, 1)))
    return np.concatenate(outs, axis=0)
